# revision 12
# baseline (speedup 1.0000x reference)
"""DigitCaps (capsule routing) Trainium2 kernel.

Self-contained: hardcodes shapes for
  x: [256, 32, 8, 6, 6] f32, W: [1, 10, 1152, 16, 8] f32 -> v: [256, 10, 16] f32

Sharding: pure data parallelism over batch, 32 batch items per core on 8
cores, processed as 4 octet groups per core.

Per-core layout: partition p = (i16, b8) (16 in-capsule offsets x 8 batch
items of the octet group), free = (ic=72, h=10, w=17); w slot 16 is a ones
column so the delta-selection matmul that computes s_un = sum_i c*u also
yields the softmax denominator d = sum_i c in the same pass.  u = W@x comes
from block-diagonal packed fp16 matmuls (K=128 fully used: lhsT =
host-built block-diag x tile, rhs = repacked W), copied PSUM->SBUF as fp16
by ScalarE.  Routing: c' = exp(logits - M_bh) with per-(b,h) max M (found
via a free-dim reduce + a partition-folding DMA bounce) keeps c' in [0,1]
so the whole mul/reduce pipeline is fp16-safe; the e^{-M} scale cancels in
s = s_un/d.  DVE does the elementwise muls (agreement mul in 2x mode);
sum_i runs on TensorE (N=510 fp16 single-pass matmuls + tiny 3-way fold);
sum_w agreement reduce is a contiguous DVE reduce-X.  sqrt in squash is
exp(0.5*ln(x)) so ACT stays on one table set.  Iteration 3's agreement
update is dead code, skipped.
"""

import numpy as np

# ---- problem constants (hardcoded) ----
B_FULL = 256
N_CORES = 8
B_CORE = B_FULL // N_CORES          # 32
NGRP = 4                            # octet groups per core
B8 = 8                              # batch per group
H = 10
WD = 16
W17 = 17                            # +1 ones column
HW = H * WD                         # 160
S = 8
NI = 1152
I16 = 16
IC = NI // I16                      # 72
ICC = 9                             # ic per W-stream chunk
NQ = IC // ICC                      # 8 W chunks
CPY = 3                             # ic per psum copy tile
CKS = 24                            # ic per s-pass mul chunk
CKA = 8                             # ic per a-pass mul chunk
P = 128

_CACHE = {}


def _build_program(debug: bool):
    import concourse.bacc as bacc
    import concourse.bass as bass
    import concourse.tile as tile
    from concourse import mybir

    f32 = mybir.dt.float32
    f16 = mybir.dt.float16
    AX = mybir.AxisListType
    AF = mybir.ActivationFunctionType

    if not getattr(bacc, "_digitcaps_act_pin", False):
        _orig_gat = bacc.get_activation_tables

        def _pinned_gat(arch):
            tables = dict(_orig_gat(arch))
            both = {mybir.ActivationFunctionType.Exp, mybir.ActivationFunctionType.Ln}
            for name in tables:
                if name != "natural_log_exp_and_others" and both & tables[name]:
                    tables[name] = tables[name] - both
            return tables

        bacc.get_activation_tables = _pinned_gat
        bacc._digitcaps_act_pin = True

    nc = bacc.Bacc(
        "TRN2", target_bir_lowering=False, debug=debug, enable_asserts=False
    )

    xd_d = nc.dram_tensor("xdiag", [NGRP, P, IC * P], f16, kind="ExternalInput")
    w_d = nc.dram_tensor("wpack", [IC, P, HW], f16, kind="ExternalInput")
    sd_d = nc.dram_tensor("sdelta", [P, B8], f16, kind="ExternalInput")
    sr_d = nc.dram_tensor("srepl", [B8, P], f32, kind="ExternalInput")
    out_d = nc.dram_tensor("vout", [B_CORE, HW], f32, kind="ExternalOutput")

    with tile.TileContext(nc) as tc:
        with (
            tc.tile_pool(name="const", bufs=1) as const_pool,
            tc.tile_pool(name="xd", bufs=1) as xd_pool,
            tc.tile_pool(name="u", bufs=4) as u_pool,
            tc.tile_pool(name="wq", bufs=2) as w_pool,
            tc.tile_pool(name="scr", bufs=2) as scr_pool,
            tc.tile_pool(name="logits", bufs=4) as log_pool,
            tc.tile_pool(name="cexp", bufs=2) as c_pool,
            tc.tile_pool(name="small", bufs=2) as small_pool,
            tc.tile_pool(name="psum_u", bufs=4, space="PSUM") as psum_u,
            tc.tile_pool(name="psum_r", bufs=2, space="PSUM") as psum_r,
        ):
            sdelta = const_pool.tile([P, B8], f16, tag="sdelta")
            nc.sync.dma_start(sdelta[:], sd_d[:])
            srepl = const_pool.tile([B8, P], f32, tag="srepl")
            nc.sync.dma_start(srepl[:], sr_d[:])

            def ucompute(g):
                xd = xd_pool.tile([P, IC, P], f16, tag="xd")
                nc.sync.dma_start(xd[:], xd_d[g].rearrange("p (ic m) -> p ic m", ic=IC))
                u = u_pool.tile([P, IC, H, W17], f16, tag="u")
                nc.vector.memset(u[:, :, :, WD], 1.0)  # ones column
                for q in range(NQ):
                    wq = w_pool.tile([P, ICC, HW], f16, tag="wq")
                    nc.sync.dma_start(
                        wq[:],
                        w_d[q * ICC : (q + 1) * ICC].rearrange("ic p f -> p ic f"),
                    )
                    for j in range(0, ICC, CPY):
                        ps = psum_u.tile([P, CPY, HW], f32, tag="ups")
                        for t in range(CPY):
                            nc.tensor.matmul(
                                ps[:, t, :],
                                xd[:, q * ICC + j + t, :],
                                wq[:, j + t, :],
                                start=True,
                                stop=True,
                            )
                        ic0 = q * ICC + j
                        nc.scalar.copy(
                            u[:, ic0 : ic0 + CPY, :, 0:WD],
                            ps[:].rearrange("p a (h w) -> p a h w", h=H),
                        )
                return u

            def routing_step(g, it, u, logits):
                cexp = None
                if it > 0:
                    # per-(b,h) max logit -> fp16-safe unnormalized softmax;
                    # shift applied in place (softmax is shift-invariant and
                    # later shifts self-correct).
                    mh = small_pool.tile([P, H], f32, tag="mh")
                    nc.vector.reduce_max(
                        mh[:], logits[:].rearrange("p i h -> p h i"), axis=AX.X
                    )
                    mt = small_pool.tile([1, P * H], f32, tag="mt")
                    nc.sync.dma_start(mt[:], mh[:])
                    mbh = small_pool.tile([1, B8 * H], f32, tag="mbh")
                    mtv = mt[:].rearrange("q (i b h) -> q b h i", i=I16, b=B8)
                    nc.vector.reduce_max(
                        mbh[:].rearrange("q (b h) -> q b h", b=B8), mtv, axis=AX.X
                    )
                    nc.vector.tensor_scalar_mul(mbh[:], mbh[:], -1.0)
                    mrep = small_pool.tile([1, P * H], f32, tag="mrep")
                    nc.vector.tensor_copy(
                        mrep[:].rearrange("q (i b h) -> q i b h", i=I16, b=B8),
                        mbh[:]
                        .rearrange("q (b h) -> q b h", b=B8)
                        .unsqueeze(1)
                        .to_broadcast([1, I16, B8, H]),
                    )
                    negm = small_pool.tile([P, H], f32, tag="negm")
                    nc.sync.dma_start(negm[:], mrep[:])
                    nc.vector.tensor_add(
                        logits[:],
                        logits[:],
                        negm[:].unsqueeze(1).to_broadcast([P, IC, H]),
                    )
                    cexp = c_pool.tile([P, IC, H], f16, tag="cexp")
                    nc.scalar.activation(cexp[:], logits[:], AF.Exp)

                # s_un (+d in w slot 16) = sum_i c_un * u  on TensorE
                sun = psum_r.tile([B8, H, W17], f32, tag="sps")
                if it == 0:
                    for j in range(IC):
                        nc.tensor.matmul(
                            sun[:],
                            sdelta[:],
                            u[:, j],
                            start=(j == 0),
                            stop=(j == IC - 1),
                        )
                else:
                    for qi, c0 in enumerate(range(0, IC, CKS)):
                        pr = scr_pool.tile([P, CKS, H, W17], f16, tag="prs")
                        cb = (
                            cexp[:, c0 : c0 + CKS]
                            .unsqueeze(3)
                            .to_broadcast([P, CKS, H, W17])
                        )
                        eng = nc.gpsimd if qi == 2 else nc.vector
                        eng.tensor_mul(pr[:], u[:, c0 : c0 + CKS], cb)
                        for j in range(CKS):
                            ic = c0 + j
                            nc.tensor.matmul(
                                sun[:],
                                sdelta[:],
                                pr[:, j],
                                start=(ic == 0),
                                stop=(ic == IC - 1),
                            )

                # ---- squash: s = s_un/d; v = s*sqrt(sq)/(1+sq) ----
                dinv = small_pool.tile([B8, H], f32, tag="dinv")
                nc.vector.reciprocal(dinv[:], sun[:, :, WD])
                s = small_pool.tile([B8, H, WD], f32, tag="s")
                db = dinv[:].unsqueeze(2).to_broadcast([B8, H, WD])
                nc.vector.tensor_mul(s[:], sun[:, :, 0:WD], db)
                s2 = small_pool.tile([B8, H, WD], f32, tag="s2")
                nc.vector.tensor_mul(s2[:], s[:], s[:])
                sq = small_pool.tile([B8, H], f32, tag="sq")
                nc.vector.reduce_sum(sq[:], s2[:], axis=AX.X)
                # sqrt(sq) = exp(0.5*ln(sq)); Exp+Ln share one table set
                lg = small_pool.tile([B8, H], f32, tag="lg")
                nc.scalar.activation(lg[:], sq[:], AF.Ln)
                rt = small_pool.tile([B8, H], f32, tag="rt")
                nc.scalar.activation(rt[:], lg[:], AF.Exp, scale=0.5)
                onep = small_pool.tile([B8, H], f32, tag="onep")
                nc.vector.tensor_scalar_add(onep[:], sq[:], 1.0)
                rr = small_pool.tile([B8, H], f32, tag="rr")
                nc.vector.reciprocal(rr[:], onep[:])
                f = small_pool.tile([B8, H], f32, tag="f")
                nc.vector.tensor_mul(f[:], rt[:], rr[:])
                v = small_pool.tile([B8, H, WD], f32, tag="v")
                fb = f[:].unsqueeze(2).to_broadcast([B8, H, WD])
                nc.vector.tensor_mul(v[:], s[:], fb)

                if it == 2:
                    nc.sync.dma_start(
                        out_d[g * B8 : (g + 1) * B8, :],
                        v[:].rearrange("b h w -> b (h w)"),
                    )
                    return
                # broadcast v across partitions via PE; cast to fp16
                vb = psum_r.tile([P, HW], f32, tag="vb")
                nc.tensor.matmul(
                    vb[:],
                    srepl[:],
                    v[:].rearrange("b h w -> b (h w)"),
                    start=True,
                    stop=True,
                )
                vb16 = small_pool.tile([P, H, WD], f16, tag="vb16")
                nc.scalar.copy(vb16[:], vb[:].rearrange("p (h w) -> p h w", h=H))
                # agreement: logits[:, ic, :] (+)= sum_w u*v
                for c0 in range(0, IC, CKA):
                    pr = scr_pool.tile([P, CKA, H, WD], f16, tag="pra")
                    vbb = vb16[:].unsqueeze(1).to_broadcast([P, CKA, H, WD])
                    nc.vector.tensor_mul(pr[:], u[:, c0 : c0 + CKA, :, 0:WD], vbb)
                    if it == 0:
                        nc.vector.reduce_sum(
                            logits[:, c0 : c0 + CKA], pr[:], axis=AX.X
                        )
                    else:
                        at = small_pool.tile([P, CKA, H], f32, tag="at")
                        nc.vector.reduce_sum(at[:], pr[:], axis=AX.X)
                        nc.vector.tensor_add(
                            logits[:, c0 : c0 + CKA],
                            logits[:, c0 : c0 + CKA],
                            at[:],
                        )

            us, lgs = [], []
            for g in range(NGRP):
                u = ucompute(g)
                logits = log_pool.tile([P, IC, H], f32, tag="logits")
                us.append(u)
                lgs.append(logits)
                routing_step(g, 0, u, logits)
            for it in (1, 2):
                for g in range(NGRP):
                    routing_step(g, it, us[g], lgs[g])

    nc.compile()
    return nc


def _host_inputs(x: np.ndarray, W: np.ndarray):
    """Build per-core input maps."""
    xr = np.ascontiguousarray(x.reshape(B_FULL, NI, S).astype(np.float32, copy=False))
    W0 = np.asarray(W, dtype=np.float32).reshape(H, NI, WD, S)
    # wpack[ic, (i16,s), (h,w)] = W0[h, ic*16+i16, w, s]
    wpack = np.ascontiguousarray(
        W0.reshape(H, IC, I16, WD, S)
        .transpose(1, 2, 4, 0, 3)
        .reshape(IC, P, HW)
        .astype(np.float16)
    )
    # sdelta[p, b'] = (p % 8 == b');  srepl = sdelta.T
    pidx = np.arange(P)
    sdelta = (pidx[:, None] % B8 == np.arange(B8)[None, :]).astype(np.float16)
    srepl = np.ascontiguousarray(
        (pidx[None, :] % B8 == np.arange(B8)[:, None]).astype(np.float32)
    )

    in_maps = []
    for c in range(N_CORES):
        xc = xr[c * B_CORE : (c + 1) * B_CORE]  # [32, 1152, 8]
        # xdiag[g, (i16,s), ic*128 + i16*8 + b] = xc[g*8+b, ic*16+i16, s]
        xd = np.zeros((NGRP, P, IC, I16, B8), dtype=np.float16)
        xg = xc.reshape(NGRP, B8, IC, I16, S).astype(np.float16)
        for k in range(I16):
            xd[:, k * S : (k + 1) * S, :, k, :] = xg[:, :, :, k, :].transpose(
                0, 3, 2, 1
            )
        in_maps.append(
            {
                "xdiag": np.ascontiguousarray(xd.reshape(NGRP, P, IC * P)),
                "wpack": wpack,
                "sdelta": sdelta,
                "srepl": srepl,
            }
        )
    return in_maps


def kernel(x: np.ndarray, W: np.ndarray) -> np.ndarray:
    from concourse import bass_utils

    if "nc" not in _CACHE:
        _CACHE["nc"] = _build_program(debug=False)
    nc = _CACHE["nc"]
    in_maps = _host_inputs(x, W)
    res = bass_utils.run_bass_kernel_spmd(nc, in_maps, list(range(N_CORES)))
    outs = [res.results[c]["vout"].reshape(B_CORE, H, WD) for c in range(N_CORES)]
    return np.concatenate(outs, axis=0).astype(np.float32)
